# revision 21
# baseline (speedup 1.0000x reference)
"""AFM (attentional factorization machine) Trainium2 kernel, v2.

Full inputs in, full output out. Shards batch (4096) across 8 cores
(512 rows each), one SPMD Bass graph, gathers.

Math per batch row b (F=50 fields, D=16, A=32, P=1225 pairs):
  inter[p,:] = e_i * e_j                 (elementwise)
  z_a        = inter . W[:,a]
  score_p    = sum_a h_a relu(z_a)
             = 0.5*inter.(W h) + 0.5*(sum_{a:h>=0}|zt_a| - sum_{a:h<0}|zt_a|)
               where zt_a = inter.(W[:,a] h_a)       [relu eliminated]
  out        = (sum_p exp(score_p - max) * (inter.p_vec)) / sum_p exp(...)

Mapping:
  - Host pre-transposes feat to featT[(f%8)*16+d ; f//8, b] bf16 for
    fields 0..47, plus fields 48/49 replicated across the 8 partition
    blocks (rep48/rep49).  48 = 8*6 exactly, so a mod-48 rotation
    pairing (f, (f+delta) mod 48), delta=1..24, enumerates the 1128
    pairs among fields 0..47 with rectangular DVE ops; the 97 pairs
    involving fields 48/49 use the rep tensors.  1225 pairs map to a
    slot grid (g=0..7, c=0..153) with 7 dummy slots at (g>=1, c=153).
  - DVE builds interT[(g,d)=128 ; c=154, b] bf16 (pair products).
  - TensorE, per 128-row chunk and per c: stationary = interT[:,c,b128]
    (K=128=(g,d), M=128=b), moving wmov [128,256] = block-diag of
    0.5*W*h columns (pos-h cols first) -> zt PSUM [b; g,32], plus a
    second N=16 matmul wcp -> (0.5*inter.Wh, inter.p) per slot.
  - DVE reduces |zt| directly from PSUM with grouped abs-reduces
    (tensor_reduce X, apply_absolute_value), 6 tiles per op.
  - score = Rpos - Rneg + cdot; softmax via ACT exp (accum_out gives
    the denominator); numerator via tensor_tensor_reduce with pdot.

PSUM banks: z tiles 2-per-bank (even slot start=True clears the bank,
odd slot start=False overwrites its clean half), 6 banks rotating with
6-tile reduce groups, 2 groups in flight; cp tiles 32-per-bank in the
other 2 banks, evacuated by ScalarE in 32-tile groups.
"""

import sys

sys.path.insert(0, "/opt/trn_rl_repo")

import numpy as np

F = 50
D = 16
A = 32
B = 4096
NCORES = 8
BL = B // NCORES      # 512 rows per core
NT = 6                # t-blocks of 8 fields (fields 0..47)
NCOL = 154            # slot columns
NR = 2                # interT build rounds
BR = BL // NR         # 256 batch rows per build round
CH = 128              # rows per matmul chunk
CPR = BR // CH        # chunks per round (2)
NCHUNK = NR * CPR     # 4 chunks total
ZSLOTS = 12           # z psum tile slots (6 banks x 2)
ZG = 6                # z tiles per reduce group
NZG = (NCOL + ZG - 1) // ZG          # 26 z groups per chunk
CPSLOTS = 64          # cp psum tile slots (2 banks x 32)
CPG = 32              # cp tiles per ACT copy group
NCPG = (NCOL + CPG - 1) // CPG       # 5 cp groups per chunk
NEG_INF = -1.0e30
import os as _os
SAFE_MODE = _os.environ.get("KERNEL_SAFE", "1") == "1"


def _gen_ops():
    """Pair-product op list for the DVE interT build.

    All ops span identical partition ranges on in0/in1/out (a hardware
    requirement for InstTensorTensor).  in1 comes from a host-built
    rotation tensor rot{r} with rot_r[(fl,d), t, b] = feat field
    ((8t+fl+r) mod 48), so pair (f, (f+delta) mod 48) with delta=8k+r is
    featT[:, t] * rot_r[:, t+k] (t+k mod 6 splits into <= 2 rect ops).

    Each op: out[:, c0+i, :] = featT[:, t0+i, :] * src1[:, t1+i, :].
    """
    ops = []

    def add(s1, t0, t1, n, c0, pn=128):
        if n <= 0:
            return
        ops.append(dict(s1=s1, t0=t0, t1=t1, n=n, c0=c0, pn=pn))

    for delta in range(1, 24):
        k, r = divmod(delta, 8)
        c0 = 6 * (delta - 1)
        s1 = 'f' if r == 0 else f'rot{r}'
        if k == 0:
            add(s1, 0, 0, 6, c0)
        else:
            add(s1, 0, k, 6 - k, c0)
            add(s1, 6 - k, 0, k, c0 + 6 - k)
    add('f', 0, 3, 3, 138)           # delta = 24 (half range, in1 t+3)
    add('r48', 0, 0, 6, 141)         # (f, 48)
    add('r49', 0, 0, 6, 147)         # (f, 49)
    add('rr', 0, 0, 1, 153)          # (48,49) on all 128 partitions; g>=1 masked

    # validate: every unordered pair exactly once (except (48,49) which
    # lands on all 8 g-blocks of col 153; g>=1 are masked later)
    def fld(s1, fl, t):
        if s1 == 'f':
            return 8 * t + fl
        if s1.startswith('rot'):
            return (8 * t + fl + int(s1[3:])) % 48
        return 48 if s1 == 'r48' else 49

    seen_pairs = set()
    seen_slots = set()
    for op in ops:
        for fl in range(8):
            for i in range(op['n']):
                if op['s1'] == 'rr':
                    fi, fj = 48, 49
                else:
                    fi = 8 * (op['t0'] + i) + fl
                    fj = fld(op['s1'], fl, op['t1'] + i)
                slot = (fl, op['c0'] + i)
                assert slot not in seen_slots, f"slot collision {slot}"
                seen_slots.add(slot)
                pr = (min(fi, fj), max(fi, fj))
                assert fi != fj, f"self pair at {slot}"
                if slot[1] == 153 and slot[0] >= 1:
                    assert pr == (48, 49)      # masked duplicate
                    continue
                assert pr not in seen_pairs, f"dup pair {pr} at {slot}"
                seen_pairs.add(pr)
    want = {(i, j) for i in range(F) for j in range(i + 1, F)}
    assert seen_pairs == want and len(seen_slots) == 1232
    return ops


_OPS = _gen_ops()


def _build_graph(W, h, p_vec, repeat=1):
    import concourse.bass as bass
    import concourse.mybir as mybir

    f32 = mybir.dt.float32
    bf16 = mybir.dt.bfloat16
    Alu = mybir.AluOpType
    Act = mybir.ActivationFunctionType

    npos = int((np.asarray(h, np.float64) >= 0).sum())
    nneg = A - npos

    nc = bass.Bass()
    featT_ext = nc.declare_dram_parameter("featT", [128, NT * BL], bf16, isOutput=False)
    rot_ext = [
        nc.declare_dram_parameter(f"rot{r}", [128, NT * BL], bf16, isOutput=False)
        for r in range(1, 8)
    ]
    rep48_ext = nc.declare_dram_parameter("rep48", [128, BL], bf16, isOutput=False)
    rep49_ext = nc.declare_dram_parameter("rep49", [128, BL], bf16, isOutput=False)
    wmov_ext = nc.declare_dram_parameter("wmov", [128, 256], bf16, isOutput=False)
    wcp_ext = nc.declare_dram_parameter("wcp", [128, 16], bf16, isOutput=False)
    out_ext = nc.declare_dram_parameter("out", [BL, 1], f32, isOutput=True)

    NDMA_IN = 12

    from contextlib import ExitStack

    with ExitStack() as ctx:
        block = ctx.enter_context(nc.Block())
        dma_sem = ctx.enter_context(nc.semaphore("dma_sem"))
        build_sem = ctx.enter_context(nc.semaphore("build_sem"))
        zmm_sem = ctx.enter_context(nc.semaphore("zmm_sem"))
        cpmm_sem = ctx.enter_context(nc.semaphore("cpmm_sem"))
        zfree_sem = ctx.enter_context(nc.semaphore("zfree_sem"))
        cpfree_sem = ctx.enter_context(nc.semaphore("cpfree_sem"))
        d2a_sem = ctx.enter_context(nc.semaphore("d2a_sem"))
        a2d_sem = ctx.enter_context(nc.semaphore("a2d_sem"))
        fin_sem = ctx.enter_context(nc.semaphore("fin_sem"))
        featT = ctx.enter_context(nc.sbuf_tensor("featT_s", [128, NT, BL], bf16))
        rot = [
            ctx.enter_context(nc.sbuf_tensor(f"rot{r}_s", [128, NT, BL], bf16))
            for r in range(1, 8)
        ]
        rep48 = ctx.enter_context(nc.sbuf_tensor("rep48_s", [128, BL], bf16))
        rep49 = ctx.enter_context(nc.sbuf_tensor("rep49_s", [128, BL], bf16))
        wmov = ctx.enter_context(nc.sbuf_tensor("wmov_s", [128, 256], bf16))
        wcp = ctx.enter_context(nc.sbuf_tensor("wcp_s", [128, 16], bf16))
        interT = ctx.enter_context(nc.sbuf_tensor("interT", [128, NCOL, BR], bf16))
        Rpos = ctx.enter_context(nc.sbuf_tensor("Rpos", [128, NCOL, 8], f32))
        Rneg = ctx.enter_context(nc.sbuf_tensor("Rneg", [128, NCOL, 8], f32))
        sc_a = ctx.enter_context(nc.sbuf_tensor("sc_a", [128, NCOL, 8], f32))
        sc_b = ctx.enter_context(nc.sbuf_tensor("sc_b", [128, NCOL, 8], f32))
        cp_sb = ctx.enter_context(nc.sbuf_tensor("cp_sb", [128, NCOL, 8, 2], f32))
        wexp = ctx.enter_context(nc.sbuf_tensor("wexp", [128, NCOL, 8], f32))
        scr = ctx.enter_context(nc.sbuf_tensor("scr", [128, NCOL, 8], f32))
        stat = ctx.enter_context(nc.sbuf_tensor("stat", [128, 8], f32))
        res = ctx.enter_context(nc.sbuf_tensor("res", [128, NCHUNK], f32))
        zps = ctx.enter_context(nc.psum_tensor("zps", [128, ZSLOTS, 8, 32], f32))
        cpps = ctx.enter_context(nc.psum_tensor("cpps", [128, CPSLOTS, 8, 2], f32))

        @block.sync
        def _(sp):
            sp.dma_start(out=featT[:, :, :], in_=featT_ext[:, :]).then_inc(dma_sem, 16)
            for r in range(7):
                sp.dma_start(out=rot[r][:, :, :],
                             in_=rot_ext[r][:, :]).then_inc(dma_sem, 16)
            sp.dma_start(out=rep48[:, :], in_=rep48_ext[:, :]).then_inc(dma_sem, 16)
            sp.dma_start(out=rep49[:, :], in_=rep49_ext[:, :]).then_inc(dma_sem, 16)
            sp.dma_start(out=wmov[:, :], in_=wmov_ext[:, :]).then_inc(dma_sem, 16)
            sp.dma_start(out=wcp[:, :], in_=wcp_ext[:, :]).then_inc(dma_sem, 16)
            sp.wait_ge(fin_sem, NCHUNK * repeat)
            for gbc in range(NCHUNK):
                sp.dma_start(
                    out=out_ext[CH * gbc : CH * (gbc + 1), :],
                    in_=res[:, gbc : gbc + 1],
                ).then_inc(dma_sem, 16)
            sp.wait_ge(dma_sem, 16 * (NDMA_IN + NCHUNK))

        @block.vector
        def _(v):
            v.wait_ge(dma_sem, 16 * NDMA_IN)
            gchunk = 0
            for rep in range(repeat):
                for rnd in range(NR):
                    # gate: previous round's interT fully consumed by TensorE
                    consumed = (rep * NR + rnd) * CPR * NCPG
                    if consumed > 0:
                        v.wait_ge(cpmm_sem, consumed)
                    roff = rnd * BR
                    for oi, op in enumerate(_OPS):
                        n, c0 = op['n'], op['c0']
                        s1 = op['s1']
                        if s1 == 'rr':
                            in0 = rep48[:, roff:roff + BR] \
                                .unsqueeze(1).broadcast_to((128, n, BR))
                            in1 = rep49[:, roff:roff + BR] \
                                .unsqueeze(1).broadcast_to((128, n, BR))
                        else:
                            in0 = featT[:, op['t0']:op['t0'] + n,
                                        roff:roff + BR]
                            if s1 == 'f':
                                in1 = featT[:, op['t1']:op['t1'] + n,
                                            roff:roff + BR]
                            elif s1.startswith('rot'):
                                in1 = rot[int(s1[3:]) - 1][
                                    :, op['t1']:op['t1'] + n, roff:roff + BR]
                            else:
                                t1 = rep48 if s1 == 'r48' else rep49
                                in1 = t1[:, roff:roff + BR] \
                                    .unsqueeze(1).broadcast_to((128, n, BR))
                        ins = v.tensor_tensor(
                            out=interT[:, c0:c0 + n, :],
                            in0=in0, in1=in1, op=Alu.mult)
                    ins.then_inc(build_sem, 1)

                    for bc in range(CPR):
                        # consume z tiles: grouped |.| reduces from PSUM
                        zmm_base = gchunk * NZG
                        for jg in range(NZG):
                            tiles = min(ZG, NCOL - jg * ZG)
                            v.wait_ge(zmm_sem, zmm_base + jg + 1)
                            s0 = (jg * ZG) % ZSLOTS
                            o_p = Rpos[:, jg * ZG:jg * ZG + tiles, :]
                            o_n = Rneg[:, jg * ZG:jg * ZG + tiles, :]
                            if npos > 0:
                                last = v.tensor_reduce(
                                    out=o_p,
                                    in_=zps[:, s0:s0 + tiles, :, 0:npos],
                                    axis=mybir.AxisListType.X,
                                    op=Alu.add, apply_absolute_value=True)
                            else:
                                last = v.memset(o_p, 0.0)
                            if nneg > 0:
                                last = v.tensor_reduce(
                                    out=o_n,
                                    in_=zps[:, s0:s0 + tiles, :, npos:32],
                                    axis=mybir.AxisListType.X,
                                    op=Alu.add, apply_absolute_value=True)
                            else:
                                last = v.memset(o_n, 0.0)
                            last.then_inc(zfree_sem, 1)

                        # score = Rpos - Rneg + cdot ; softmax prep
                        v.wait_ge(cpfree_sem, (gchunk + 1) * NCPG)
                        v.drain()
                        v.tensor_tensor(out=sc_a[:, :, :], in0=Rpos[:, :, :],
                                        in1=Rneg[:, :, :], op=Alu.subtract)
                        v.drain()
                        v.tensor_tensor(out=sc_b[:, :, :], in0=sc_a[:, :, :],
                                        in1=cp_sb[:, :, :, 0:1].squeeze(3),
                                        op=Alu.add)
                        v.drain()
                        v.memset(sc_b[:, 153:154, 1:8], NEG_INF)
                        v.drain()
                        v.tensor_reduce(out=stat[:, 0:1], in_=sc_b[:, :, :],
                                        axis=mybir.AxisListType.XY, op=Alu.max)
                        v.drain()
                        v.tensor_scalar_mul(stat[:, 1:2], stat[:, 0:1],
                                            -1.0).then_inc(d2a_sem, 1)

                        # numerator / denominator / output
                        v.wait_ge(a2d_sem, gchunk + 1)
                        if SAFE_MODE:
                            v.tensor_tensor(out=scr[:, :, :],
                                            in0=wexp[:, :, :],
                                            in1=cp_sb[:, :, :, 1:2].squeeze(3),
                                            op=Alu.mult)
                            v.drain()
                            v.tensor_reduce(out=stat[:, 2:3],
                                            in_=scr[:, :, :],
                                            axis=mybir.AxisListType.XY,
                                            op=Alu.add)
                            v.tensor_reduce(out=stat[:, 4:5],
                                            in_=wexp[:, :, :],
                                            axis=mybir.AxisListType.XY,
                                            op=Alu.add)
                        else:
                            v.tensor_tensor_reduce(
                                out=scr[:, :, :], in0=wexp[:, :, :],
                                in1=cp_sb[:, :, :, 1:2].squeeze(3),
                                scale=1.0, scalar=0.0,
                                op0=Alu.mult, op1=Alu.add,
                                accum_out=stat[:, 2:3])
                        v.drain()
                        v.reciprocal(stat[:, 3:4], stat[:, 4:5])
                        v.drain()
                        gbc = rnd * CPR + bc
                        v.tensor_tensor(out=res[:, gbc:gbc + 1],
                                        in0=stat[:, 2:3], in1=stat[:, 3:4],
                                        op=Alu.mult).then_inc(fin_sem, 1)
                        gchunk += 1

        @block.tensor
        def _(t):
            t.wait_ge(dma_sem, 16 * NDMA_IN)
            gchunk = 0
            for rep in range(repeat):
                for rnd in range(NR):
                    t.wait_ge(build_sem, rep * NR + rnd + 1)
                    for bc in range(CPR):
                        for c in range(NCOL):
                            if c % ZG == 0:
                                J = gchunk * NZG + c // ZG
                                if J >= 2:
                                    t.wait_ge(zfree_sem, J - 1)
                            if c in (0, 64, 96, 128):
                                need = gchunk * NCPG + {0: 0, 64: 1,
                                                        96: 2, 128: 3}[c]
                                if need > 0:
                                    t.wait_ge(cpfree_sem, need)
                            s = c % ZSLOTS
                            lhs = interT[:, c:c + 1, bc * CH:(bc + 1) * CH]
                            mm = t.matmul(
                                out=zps[:, s, :, :], lhsT=lhs, rhs=wmov[:, :],
                                start=(s % 2 == 0), stop=True,
                                skip_group_check=True,
                            )
                            if c % ZG == ZG - 1 or c == NCOL - 1:
                                mm.then_inc(zmm_sem, 1)
                            mm = t.matmul(
                                out=cpps[:, c % CPSLOTS, :, :], lhsT=lhs,
                                rhs=wcp[:, :],
                                start=(c % CPG == 0), stop=True,
                                skip_group_check=True,
                            )
                            if c % CPG == CPG - 1 or c == NCOL - 1:
                                mm.then_inc(cpmm_sem, 1)
                        gchunk += 1

        @block.scalar
        def _(s):
            gchunk = 0
            for rep in range(repeat):
                for rnd in range(NR):
                    for bc in range(CPR):
                        # don't clobber cp_sb while DVE still reads chunk-1
                        if gchunk > 0:
                            s.wait_ge(fin_sem, gchunk)
                        cpmm_base = gchunk * NCPG
                        for gc in range(NCPG):
                            tiles = min(CPG, NCOL - gc * CPG)
                            s.wait_ge(cpmm_sem, cpmm_base + gc + 1)
                            s0 = (gc * CPG) % CPSLOTS
                            s.activation(
                                out=cp_sb[:, gc * CPG:gc * CPG + tiles, :, :],
                                in_=cpps[:, s0:s0 + tiles, :, :],
                                func=Act.Copy,
                            ).then_inc(cpfree_sem, 1)
                        # exp with fused denominator accumulation
                        s.wait_ge(d2a_sem, gchunk + 1)
                        if SAFE_MODE:
                            s.activation(
                                out=wexp[:, :, :], in_=sc_b[:, :, :],
                                func=Act.Exp, bias=stat[:, 1:2], scale=1.0,
                            ).then_inc(a2d_sem, 1)
                        else:
                            s.activation(
                                out=wexp[:, :, :], in_=sc_b[:, :, :],
                                func=Act.Exp, bias=stat[:, 1:2], scale=1.0,
                                accum_out=stat[:, 4:5],
                            ).then_inc(a2d_sem, 1)
                        gchunk += 1

    return nc


def _host_prep(feat_emb, W, h, p_vec):
    import ml_dtypes

    bf16 = ml_dtypes.bfloat16
    W = np.asarray(W, np.float32)
    h = np.asarray(h, np.float32)
    p_vec = np.asarray(p_vec, np.float32)

    Wt = W * h[None, :]
    order = np.concatenate([np.where(h >= 0)[0], np.where(h < 0)[0]])
    cols = (0.5 * Wt[:, order]).astype(np.float32)          # [16, 32]
    wmov = np.zeros((128, 256), np.float32)
    for g in range(8):
        wmov[g * 16:(g + 1) * 16, g * 32:(g + 1) * 32] = cols
    cvec = 0.5 * (W.astype(np.float64) @ h.astype(np.float64)).astype(np.float32)
    wcp = np.zeros((128, 16), np.float32)
    for g in range(8):
        wcp[g * 16:(g + 1) * 16, 2 * g] = cvec
        wcp[g * 16:(g + 1) * 16, 2 * g + 1] = p_vec
    wmov = np.ascontiguousarray(wmov.astype(bf16))
    wcp = np.ascontiguousarray(wcp.astype(bf16))

    # rot_r field index per (t, fl): (8t + fl + r) mod 48
    tt, ff = np.meshgrid(np.arange(NT), np.arange(8), indexing='ij')
    base_idx = 8 * tt + ff                       # [NT, 8]

    def pack(fe_sel):
        # fe_sel [BL, NT, 8, D] -> [128, NT*BL]
        a = fe_sel.transpose(2, 3, 1, 0)         # [8, D, NT, BL]
        return np.ascontiguousarray(a.reshape(128, NT * BL).astype(bf16))

    in_maps = []
    for i in range(NCORES):
        fe = np.asarray(feat_emb[i * BL:(i + 1) * BL], np.float32)
        m = {
            "featT": pack(fe[:, base_idx, :]),
            "rep48": np.ascontiguousarray(
                np.tile(fe[:, 48, :].T, (8, 1)).astype(bf16)),
            "rep49": np.ascontiguousarray(
                np.tile(fe[:, 49, :].T, (8, 1)).astype(bf16)),
            "wmov": wmov, "wcp": wcp,
        }
        for r in range(1, 8):
            m[f"rot{r}"] = pack(fe[:, (base_idx + r) % 48, :])
        in_maps.append(m)
    return in_maps


LAST_RESULT = None
REPEAT = 1


def kernel(feat_emb, W, h, p_vec):
    from concourse.bass_utils import run_bass_kernel_spmd

    feat_emb = np.ascontiguousarray(feat_emb, dtype=np.float32)
    nc = _build_graph(W, h, p_vec, repeat=REPEAT)
    from concourse.library_overlay import lower_extended_insts
    lower_extended_insts(nc)
    in_maps = _host_prep(feat_emb, W, h, p_vec)
    r = run_bass_kernel_spmd(nc, in_maps, core_ids=list(range(NCORES)))
    global LAST_RESULT
    LAST_RESULT = r
    out = np.concatenate([r.results[i]["out"] for i in range(NCORES)], axis=0)
    return out.astype(np.float32)


if __name__ == "__main__":
    rng = np.random.default_rng(0)
    fe = rng.standard_normal((B, F, D), dtype=np.float32)
    W_ = (rng.standard_normal((D, A)) * 0.1).astype(np.float32)
    h_ = rng.standard_normal(A).astype(np.float32)
    pv = rng.standard_normal(D).astype(np.float32)
    print(kernel(fe, W_, h_, pv)[:4])
